# revision 1
# baseline (speedup 1.0000x reference)
"""Trainium2 Bass kernel for nn_CrossAttention.

Problem: B=4, S=2048, D=512 cross-attention with 3 input streams:
  Qi, Ki, Vi = xi@Wq+bq, xi@Wk+bk, xi@Wv+bv   (i = 1..3)
  fused_xi = sum over j != i of softmax(Qi Kj^T / sqrt(512)) @ Vj
  out = concat(fused_x1..3, -1) @ Wo + bo

Sharding: 8 cores = (batch b in 0..3) x (query half in 0..1). Each core runs
an identical single-core program on its own data slice: full K/V context for
its batch, a 1024-row query block, and replicated weights.

Per-core algorithm (everything stays in "transposed" layout so no transposes
are ever materialized):
  X^T [din, s] -> Q^T[h, q] = Wq^T X^T (lhsT=Wq),  K^T[h, s],  V[s, h]
  S^T[k, q]   = (K^T chunk)^T Q^T      (contract h)
  w^T         = exp(S^T * scale)       (no row-max: |scores| <= ~8, safe in fp32)
  O^T[h, q]   = V^T w^T                (lhsT = V natural layout, contract k)
  z[q]        = sum_k w^T  (DVE partial sums + tiny ones-matmul to transpose)
  out[q, :]  += (O^T chunk)^T @ Wo_i * (1/z)[q]   per attention term, plus
  bo' = bo + 2*sum_i bv@Wo_i  folded in once (softmax rows sum to 1).

Bulk matmuls run in bf16 (full PE rate) with fp32 PSUM accumulation; the
softmax statistics (z partial sums, reciprocal) and final accumulation into
the output stay fp32.
"""

import numpy as np

B, S, DIN, DH, DOUT = 4, 2048, 512, 512, 512
P = 128
DC = DIN // P      # 4  din chunks
HT = DH // P       # 4  head tiles
ST = S // P        # 16 s tiles
KT = ST            # 16 k tiles
SC = S // 512      # 4  s chunks of 512
QW = 1024          # queries per core
QC = QW // 512     # 2  query chunks of 512
SCALE = 1.0 / float(np.sqrt(DH))

_CACHE = {}


def _build_program(loop_n=1):
    import contextlib

    import concourse.bacc as bacc
    import concourse.bass_isa as bass_isa
    import concourse.library_config as library_config
    import concourse.mybir as mybir
    import concourse.tile as tile

    dt = mybir.dt
    F32 = dt.float32
    BF16 = dt.bfloat16
    AF = mybir.ActivationFunctionType
    OP = mybir.AluOpType

    nc = bacc.Bacc("TRN2", target_bir_lowering=False, debug=False, num_devices=8)

    xT = [
        nc.dram_tensor(f"xT{i}", [DIN, S], BF16, kind="ExternalInput").ap()
        for i in range(3)
    ]
    Wq_d = nc.dram_tensor("Wq", [DIN, DH], BF16, kind="ExternalInput").ap()
    Wk_d = nc.dram_tensor("Wk", [DIN, DH], BF16, kind="ExternalInput").ap()
    Wv_d = nc.dram_tensor("Wv", [DIN, DH], BF16, kind="ExternalInput").ap()
    Wo_d = nc.dram_tensor("Wo", [3 * DH, DOUT], BF16, kind="ExternalInput").ap()
    bq_d = nc.dram_tensor("bq", [DH], F32, kind="ExternalInput").ap()
    bk_d = nc.dram_tensor("bk", [DH], F32, kind="ExternalInput").ap()
    bv_d = nc.dram_tensor("bv", [DH], BF16, kind="ExternalInput").ap()
    bo_d = nc.dram_tensor("bo", [DOUT], F32, kind="ExternalInput").ap()
    out_d = nc.dram_tensor("out", [QW, DOUT], F32, kind="ExternalOutput").ap()

    def mm(out, lhsT, rhs, start, stop):
        assert lhsT.dtype == rhs.dtype, (lhsT.dtype, rhs.dtype)
        nc.tensor.matmul(out, lhsT, rhs, start=start, stop=stop)

    with tile.TileContext(nc) as tc:
        with (
            tc.tile_pool(name="const", bufs=1) as cpool,
            tc.tile_pool(name="kv", bufs=1) as kvpool,
            tc.tile_pool(name="qslota", bufs=1) as qapool,
            tc.tile_pool(name="qslotb", bufs=1) as qbpool,
            tc.tile_pool(name="qslotc", bufs=1) as qcpool,
            tc.tile_pool(name="xin", bufs=2) as xpool,
            tc.tile_pool(name="wts", bufs=3) as wtpool,
            tc.tile_pool(name="osb", bufs=2) as opool,
            tc.tile_pool(name="zps", bufs=2) as zppool,
            tc.tile_pool(name="accp", bufs=1) as accpool,
            tc.tile_pool(name="zsums", bufs=2) as zsumpool,
            tc.tile_pool(name="rbp", bufs=2) as rbpool,
            tc.tile_pool(name="fusedp", bufs=6) as fusedpool,
            tc.tile_pool(name="tmpf", bufs=2) as tmppool,
            tc.tile_pool(name="ps", bufs=4, space="PSUM") as pspool,
            tc.tile_pool(name="pso", bufs=1, space="PSUM") as psopool,
        ):
            # partition_all_reduce lives in the gpsimd "attn" ucode library
            nc.gpsimd.load_library(library_config.attn)
            # ---- constants ----
            wq_sb = cpool.tile([P, DC, DH], BF16, name="wq_sb")
            wk_sb = cpool.tile([P, DC, DH], BF16, name="wk_sb")
            wv_sb = cpool.tile([P, DC, DH], BF16, name="wv_sb")
            wo_sb = cpool.tile([P, 3 * HT, DOUT], BF16, name="wo_sb")
            bq_sb = cpool.tile([P, HT], F32, name="bq_sb")
            bk_sb = cpool.tile([P, HT], F32, name="bk_sb")
            bv2_sb = cpool.tile([P, HT], BF16, name="bv2_sb")
            bo_sb = cpool.tile([1, DOUT], F32, name="bo_sb")
            ones_sb = cpool.tile([P, P], F32, name="ones_sb")
            bob_sb = cpool.tile([P, DOUT], F32, name="bob_sb")

            nc.sync.dma_start(out=wq_sb[:], in_=Wq_d.rearrange("(c p) h -> p c h", p=P))
            nc.sync.dma_start(out=wk_sb[:], in_=Wk_d.rearrange("(c p) h -> p c h", p=P))
            nc.sync.dma_start(out=wv_sb[:], in_=Wv_d.rearrange("(c p) h -> p c h", p=P))
            nc.sync.dma_start(out=wo_sb[:], in_=Wo_d.rearrange("(c p) h -> p c h", p=P))
            nc.sync.dma_start(out=bq_sb[:], in_=bq_d.rearrange("(t p) -> p t", p=P))
            nc.sync.dma_start(out=bk_sb[:], in_=bk_d.rearrange("(t p) -> p t", p=P))
            nc.sync.dma_start(out=bv2_sb[:], in_=bv_d.rearrange("(t p) -> p t", p=P))
            nc.sync.dma_start(out=bo_sb[:], in_=bo_d.rearrange("(a d) -> a d", a=1))
            nc.vector.memset(ones_sb[:], 1.0)

            # bo' = bo + 2*sum_i bv @ Wo_i ; broadcast over partitions
            nc.vector.tensor_scalar_mul(bv2_sb[:], bv2_sb[:], 2.0)
            ps_bo = pspool.tile([1, DOUT], F32, name="ps_bo", tag="ps")
            n = 0
            for i in range(3):
                for c in range(DC):
                    mm(
                        ps_bo[:],
                        bv2_sb[:, c : c + 1],
                        wo_sb[:, i * HT + c, :],
                        start=(n == 0),
                        stop=(n == 11),
                    )
                    n += 1
            bo1_sb = cpool.tile([1, DOUT], F32, name="bo1_sb")
            nc.vector.tensor_add(bo1_sb[:], ps_bo[:], bo_sb[:])
            ps_bob = pspool.tile([P, DOUT], F32, name="ps_bob", tag="ps")
            mm(ps_bob[:], ones_sb[0:1, :], bo1_sb[:], start=True, stop=True)
            nc.scalar.activation(bob_sb[:], ps_bob[:], AF.Copy)

            # ---- Q^T projection into a slot ----
            def project_q(i, pool, tag):
                q_sb = pool.tile([P, HT, QW], BF16, name=f"q_{tag}")
                for qc in range(QC):
                    xc = xpool.tile([P, DC, 512], BF16, name="xq_chunk", tag="xch")
                    nc.sync.dma_start(
                        out=xc[:],
                        in_=xT[i][:, qc * 512 : (qc + 1) * 512].rearrange(
                            "(c p) s -> p c s", p=P
                        ),
                    )
                    for ht in range(HT):
                        ps = pspool.tile([P, 512], F32, name="ps_q", tag="ps")
                        for dc in range(DC):
                            mm(
                                ps[:],
                                wq_sb[:, dc, ht * P : (ht + 1) * P],
                                xc[:, dc, :],
                                start=(dc == 0),
                                stop=(dc == DC - 1),
                            )
                        nc.scalar.activation(
                            q_sb[:, ht, qc * 512 : (qc + 1) * 512],
                            ps[:],
                            AF.Identity,
                            bias=bq_sb[:, ht : ht + 1],
                        )
                return q_sb

            # ---- K^T and V projection for context j ----
            def project_kv(j):
                kT_sb = kvpool.tile([P, HT, S], BF16, name="kT_sb")
                v_sb = kvpool.tile([P, ST, DH], BF16, name="v_sb")
                for sc in range(SC):
                    xc = xpool.tile([P, DC, 512], BF16, name="xkv_chunk", tag="xch")
                    nc.sync.dma_start(
                        out=xc[:],
                        in_=xT[j][:, sc * 512 : (sc + 1) * 512].rearrange(
                            "(c p) s -> p c s", p=P
                        ),
                    )
                    for ht in range(HT):
                        ps = pspool.tile([P, 512], F32, name="ps_k", tag="ps")
                        for dc in range(DC):
                            mm(
                                ps[:],
                                wk_sb[:, dc, ht * P : (ht + 1) * P],
                                xc[:, dc, :],
                                start=(dc == 0),
                                stop=(dc == DC - 1),
                            )
                        nc.scalar.activation(
                            kT_sb[:, ht, sc * 512 : (sc + 1) * 512],
                            ps[:],
                            AF.Identity,
                            bias=bk_sb[:, ht : ht + 1],
                        )
                    for st4 in range(4):
                        st = sc * 4 + st4
                        ps = pspool.tile([P, 512], F32, name="ps_v", tag="ps")
                        for dc in range(DC):
                            mm(
                                ps[:],
                                xc[:, dc, st4 * P : (st4 + 1) * P],
                                wv_sb[:, dc, :],
                                start=(dc == 0),
                                stop=(dc == DC - 1),
                            )
                        # V without bias: bv is folded into bo'
                        nc.vector.tensor_copy(v_sb[:, st, :], ps[:])
                return kT_sb, v_sb

            # ---- attention units with a cross-unit software pipeline ----
            # One unit = (queries i vs context j) x one 512-query chunk.
            # The epilogue of unit u (PSUM->SBUF copies, z all-reduce +
            # reciprocal, normalized accumulation into fused_i, and after the
            # second pair of an i also the output projection) is emitted
            # interleaved into unit u+1's score phase so the PE never idles
            # waiting for ACT/DVE/GPSIMD epilogue work.
            fstate = {}

            def make_epilogue(i, qc, po, zp, pair_b, first_out, acc):
                state = {}

                def early():
                    # free the PV psum quickly (no data deps beyond po)
                    o_sb = opool.tile([P, HT, 512], BF16, name="o_sb")
                    for ht in range(HT):
                        if ht < 2:
                            nc.scalar.activation(
                                o_sb[:, ht, :], po[:, ht, :], AF.Copy
                            )
                        else:
                            nc.vector.tensor_copy(o_sb[:, ht, :], po[:, ht, :])
                    # z[q] broadcast across partitions via gpsimd all-reduce
                    zsum = zsumpool.tile([P, 512], F32, name="zsum")
                    nc.gpsimd.partition_all_reduce(
                        zsum[:], zp[:], P, bass_isa.ReduceOp.add
                    )
                    rb = rbpool.tile([P, 512], F32, name="rb")
                    nc.vector.reciprocal(rb[:], zsum[:])
                    if not pair_b:
                        fp = fusedpool.tile(
                            [P, HT, 512], BF16, name="fused", tag="fused"
                        )
                        for ht in range(HT):
                            nc.vector.tensor_mul(
                                fp[:, ht, :], o_sb[:, ht, :], rb[:]
                            )
                        fstate[(i, qc)] = fp
                    else:
                        fp = fstate.pop((i, qc))
                        tmp = tmppool.tile([P, HT, 512], BF16, name="tmpf")
                        for ht in range(HT):
                            nc.vector.tensor_mul(
                                tmp[:, ht, :], o_sb[:, ht, :], rb[:]
                            )
                        for ht in range(HT):
                            nc.vector.tensor_add(
                                fp[:, ht, :], tmp[:, ht, :], fp[:, ht, :]
                            )
                        state["fp"] = fp

                def late(qs):
                    fp = state["fp"]
                    qt = qc * 4 + qs
                    py = pspool.tile([P, 512], F32, name="ps_y", tag="ps")
                    for hc in range(HT):
                        mm(
                            py[:],
                            fp[:, hc, qs * P : (qs + 1) * P],
                            wo_sb[:, i * HT + hc, :],
                            start=(hc == 0),
                            stop=(hc == HT - 1),
                        )
                    base = bob_sb[:] if first_out else acc[:, qt, :]
                    nc.vector.tensor_add(acc[:, qt, :], py[:], base)

                return {"early": early, "late": late if pair_b else None}

            def attn_unit(i, q_sb, kT_sb, v_sb, qc, epi_args, acc, prev_epi):
                po = psopool.tile([P, HT, 512], F32, name="ps_o")
                zp = zppool.tile([P, 512], F32, name="zp")
                ps_s = {}

                def s_group(kt):
                    ps = pspool.tile([P, 512], F32, name="ps_s", tag="ps")
                    for hc in range(HT):
                        mm(
                            ps[:],
                            kT_sb[:, hc, kt * P : (kt + 1) * P],
                            q_sb[:, hc, qc * 512 : (qc + 1) * 512],
                            start=(hc == 0),
                            stop=(hc == HT - 1),
                        )
                    ps_s[kt] = ps

                s_group(0)
                s_group(1)
                if prev_epi is not None:
                    prev_epi["early"]()
                for kt in range(KT):
                    if kt + 2 < KT:
                        s_group(kt + 2)
                    wt = wtpool.tile([P, 512], BF16, name="wt")
                    nc.scalar.activation(wt[:], ps_s.pop(kt)[:], AF.Exp, scale=SCALE)
                    for ht in range(HT):
                        mm(
                            po[:, ht, :],
                            v_sb[:, kt, ht * P : (ht + 1) * P],
                            wt[:],
                            start=(kt == 0),
                            stop=(kt == KT - 1),
                        )
                    if kt == 0:
                        nc.vector.tensor_copy(zp[:], wt[:])
                    else:
                        nc.vector.tensor_add(zp[:], zp[:], wt[:])
                    if (
                        prev_epi is not None
                        and prev_epi["late"] is not None
                        and 5 <= kt <= 8
                    ):
                        prev_epi["late"](kt - 5)

                pair_b, first_out = epi_args
                return make_epilogue(i, qc, po, zp, pair_b, first_out, acc)

            # ---- main schedule ----
            # loop_n > 1 repeats the whole body on-device (used only for
            # timing measurements; output is still written every iteration)
            loop_ctx = (
                tc.For_i(0, loop_n, 1) if loop_n > 1 else contextlib.nullcontext()
            )
            with loop_ctx:
                acc = accpool.tile([P, QW // P, DOUT], F32, name="acc")

                q_all = [
                    project_q(0, qapool, "a"),
                    project_q(1, qbpool, "b"),
                    project_q(2, qcpool, "c"),
                ]

                pending = None
                occur = {0: 0, 1: 0, 2: 0}
                for j in (1, 2, 0):
                    kT_sb, v_sb = project_kv(j)
                    pairs = [(i, q_all[i]) for i in range(3) if i != j]
                    for i, q_sb in pairs:
                        pair_b = occur[i] == 1
                        for qc in range(QC):
                            pending = attn_unit(
                                i, q_sb, kT_sb, v_sb, qc,
                                (pair_b, i == 0), acc, pending,
                            )
                        occur[i] += 1
                # flush the last unit's epilogue
                pending["early"]()
                for qs in range(4):
                    pending["late"](qs)

                nc.sync.dma_start(
                    out=out_d.rearrange("(t p) d -> p t d", p=P), in_=acc[:]
                )

    nc.compile()
    return nc


def _get_program():
    if "nc" not in _CACHE:
        _CACHE["nc"] = _build_program()
    return _CACHE["nc"]


def kernel(**inputs):
    import ml_dtypes

    from concourse.bass_utils import run_bass_kernel_spmd

    nc = _get_program()
    bf16 = ml_dtypes.bfloat16

    x = [np.asarray(inputs[k], np.float32) for k in ("x1", "x2", "x3")]
    common = {}
    for k in ("Wq", "Wk", "Wv", "Wo", "bv"):
        common[k] = np.ascontiguousarray(np.asarray(inputs[k], np.float32)).astype(
            bf16
        )
    for k in ("bq", "bk", "bo"):
        common[k] = np.ascontiguousarray(np.asarray(inputs[k], np.float32))

    in_maps = []
    for b in range(B):
        xTb = [np.ascontiguousarray(xi[b].T).astype(bf16) for xi in x]  # [512, 2048]
        for half in range(2):
            if half == 0:
                perm = xTb
            else:
                # query block must be the first 1024 columns; k-order is
                # irrelevant (softmax sums over k)
                perm = [
                    np.ascontiguousarray(
                        np.concatenate([t[:, QW:], t[:, :QW]], axis=1)
                    )
                    for t in xTb
                ]
            m = dict(common)
            for i in range(3):
                m[f"xT{i}"] = perm[i]
            in_maps.append(m)

    res = run_bass_kernel_spmd(nc, in_maps, core_ids=list(range(8)))

    y = np.empty((B, S, DOUT), np.float32)
    for c, r in enumerate(res.results):
        b, half = divmod(c, 2)
        y[b, half * QW : (half + 1) * QW] = r["out"]
    return y

